# revision 4
# baseline (speedup 1.0000x reference)
"""Sliding-window causal self-attention (B=2, T=2048, D=1024, H=16, dk=64, W=512)
on 8 Trainium2 NeuronCores.

Sharding: core = (b, hg) for b in {0,1}, head-group hg in {0..3}.
Data parallel over batch, tensor parallel over heads: each core gets
x[b]^T, the 4-head column slices of Wq/Wk/Wv (+bq slice) and the matching
row slice of Wo, and produces a partial [T, D] output (fp16).  Host gathers
with out[b] = sum_hg partial[b,hg] + (bv @ Wo + bo) in fp32.

Math notes (exact softmax identities, validated vs reference):
 - bk shifts every logit of a row by a per-row constant -> cancels in softmax.
 - bv enters the output linearly with weights summing to 1 -> folded into the
   host-side bias term bv @ Wo (+ bo), added once after the cross-core sum.
 - no max-subtraction in softmax: logits are O(1), fp16 exp is safe
   (|S/8| < 6 -> exp < 403 << 65504).

Precision plan (v4): measured on HW, a rotating fp16 [128,128] stationary
exposes ~40-210ns of LDWEIGHTS per matmul, while fp8 stationaries (and
64-row fp16 stationaries) hide it completely.  fp8 also halves the input
DMA.  But fp8 fails accuracy for output rows < 512 (tiny attention windows
concentrate quantization error: row r averages ~0.37*r keys, so early rows
see individual-V-element error unaveraged).  Hybrid:
 - rows/keys 0..511 ("g0"): x, W stationaries, V, osb all fp16.
 - rows 512+: x (fp8 moving+stationary), Wq/Wk/Wv (x32-prescaled fp8
   stationaries, descaled in the psum->SBUF copy), V_aug fp8 stationary,
   osb fp8 stationary.  Q, K, pt (exp), Wo moving stay fp16 everywhere —
   moving dtype doesn't change the PE stream rate, so fp16 there is free
   accuracy.  Numpy-simulated end-to-end rel err 8.9e-3 (gate 2e-2).
S stationaries (K) are 64-row fp16 -> already hidden; left fp16.

The PE p-state ramp costs ~2x for the first ~3.5us of continuous matmul
work, and the first exp pays a ~1.3us activation-table load; both are
prepaid during the DMA head with warmup matmuls / a dummy exp on a
memset tile.

Schedule (v3): both head-pairs' attention J-loops run interleaved in one
18-step pipeline (pair23 lags pair01 by 2 steps), with the Q/K/V projection
work chopped into ~2-4k-cycle units and woven between the S-matmul blocks so
the PE never starves while the ACT (exp) pipeline drains.

Step s: S(pair01, J=s) | pre-filler | S(pair23, J=s-2) | post-filler+groups.
PV groups: pair01 at s=4g+3, pair23 at s=4g+5; the output projection for
group g runs right after pair23's group g normalizes (all 4 heads ready).

The V_aug stationary carries a 64-wide ones block ahead of the 64 V
columns, so the PV matmul emits the softmax denominator already broadcast
across psum partitions 0:64 -- no denominator copy and no rank-1 broadcast
matmul (and the custom-DVE reciprocal reads psum at base partition 0; it
returns garbage on hardware at base 64).  Input DMAs are plain contiguous
per-chunk transfers (a rearranged multi-descriptor DMA's completion
semaphore was observed to fire before all bytes landed, corrupting
first-run results).
"""

import math
from contextlib import ExitStack

import numpy as np

import concourse.bass as bass
import concourse.mybir as mybir
import concourse.tile as tile
from concourse import bacc
from concourse.bass_utils import run_bass_kernel_spmd

F32 = mybir.dt.float32
F16 = mybir.dt.float16
F8 = mybir.dt.float8e4

T = 2048
D = 1024
NHEAD = 16
DK = 64
WINDOW = 512
HPC = 4            # heads per core
HCOLS = HPC * DK   # 256 projected columns per core
NJ = T // 128      # 16 j/query blocks
NKC = D // 128     # 8 contraction chunks over D
NG = 4             # query-block groups of 512
WS = 32.0          # fp8 weight prescale (W sigma = 1/32 -> sigma 1)

_NC_CACHE = {}


def _emit(tc):
    nc = tc.nc
    xT16_d = nc.dram_tensor("xT16", [D, 512], F16, kind="ExternalInput").ap()
    xT8_d = nc.dram_tensor("xT8", [D, T - 512], F8, kind="ExternalInput").ap()
    wq16_d = nc.dram_tensor("wq16", [D, HCOLS], F16, kind="ExternalInput").ap()
    wk16_d = nc.dram_tensor("wk16", [D, HCOLS], F16, kind="ExternalInput").ap()
    wv16_d = nc.dram_tensor("wv16", [D, HCOLS], F16, kind="ExternalInput").ap()
    wq8_d = nc.dram_tensor("wq8", [D, HCOLS], F8, kind="ExternalInput").ap()
    wk8_d = nc.dram_tensor("wk8", [D, HCOLS], F8, kind="ExternalInput").ap()
    wv8_d = nc.dram_tensor("wv8", [D, HCOLS], F8, kind="ExternalInput").ap()
    wo_d = nc.dram_tensor("wo", [HCOLS, D], F16, kind="ExternalInput").ap()
    bq_d = nc.dram_tensor("bqp", [128, 2], F32, kind="ExternalInput").ap()
    mlo_d = nc.dram_tensor("mlo", [128, 128], F16, kind="ExternalInput").ap()
    mhi_d = nc.dram_tensor("mhi", [128, 128], F16, kind="ExternalInput").ap()
    out_d = nc.dram_tensor("out", [T, D], F16, kind="ExternalOutput").ap()

    with ExitStack() as ctx:
        const_pool = ctx.enter_context(tc.tile_pool(name="const", bufs=1))
        qk_pool = ctx.enter_context(tc.tile_pool(name="qk", bufs=1))
        w_pool = ctx.enter_context(tc.tile_pool(name="w", bufs=1))
        xt_pool = ctx.enter_context(tc.tile_pool(name="xt", bufs=3))
        pt_pool = ctx.enter_context(tc.tile_pool(name="pt", bufs=36))
        nrm_pool = ctx.enter_context(tc.tile_pool(name="nrm", bufs=4))
        stage_pool = ctx.enter_context(tc.tile_pool(name="stage", bufs=2))
        ps_s = ctx.enter_context(tc.tile_pool(name="ps_s", bufs=2, space="PSUM"))
        ps_pv = ctx.enter_context(tc.tile_pool(name="ps_pv", bufs=2, space="PSUM"))
        ps_mi = ctx.enter_context(tc.tile_pool(name="ps_mi", bufs=2, space="PSUM"))

        bq_sb = const_pool.tile([128, 2], F32)
        mask_lo = const_pool.tile([128, 128], F16)   # keep c >= p (upper incl)
        mask_hi = const_pool.tile([128, 128], F16)   # keep c < p (strict lower)
        warm = const_pool.tile([128, 640], F16)
        expw = const_pool.tile([128, 16], F16)

        wo_sb = qk_pool.tile([128, 2, D], F16)
        # V storage [j-part, J, head, 2*dk]; cols 0:64 of each head slot
        # are 1.0, so the PV matmul emits the softmax denominator already
        # broadcast across psum partitions 0:64.  v8 covers all J (fp8
        # stationary = hidden LDWEIGHTS); v16 duplicates J0..3 for the
        # precision-critical g0 PV.
        v8_sb = qk_pool.tile([128, NJ, HPC, 2 * DK], F8)
        v16_sb = qk_pool.tile([128, 4, HPC, 2 * DK], F16)
        q_sb = qk_pool.tile([128, 2, T], F16)
        k_sb = qk_pool.tile([128, 2, T], F16)
        osb16 = qk_pool.tile([128, 2, 512], F16)   # normalized O^T, g0
        osb8 = qk_pool.tile([128, 2, T], F8)       # normalized O^T, g1..3

        wq16_sb = w_pool.tile([128, NKC, HCOLS], F16)
        wk16_sb = w_pool.tile([128, NKC, HCOLS], F16)
        wv16_sb = w_pool.tile([128, NKC, HCOLS], F16)
        wq8_sb = w_pool.tile([128, NKC, HCOLS], F8)
        wk8_sb = w_pool.tile([128, NKC, HCOLS], F8)
        wv8_sb = w_pool.tile([128, NKC, HCOLS], F8)

        # ---- PE p-state + exp-table warmup (runs inside the DMA head) ----
        nc.vector.memset(warm[:], 0.0)
        nc.scalar.activation(expw[:], warm[:, 0:16],
                             mybir.ActivationFunctionType.Exp, scale=0.125)
        wtile = ps_mi.tile([128, 512], F32, tag="mi", name="warm")
        for _ in range(8):
            nc.tensor.matmul(wtile[:], warm[:, 0:128], warm[:, 128:640],
                             start=True, stop=True)

        # ---- x^T streamed by 512-column blocks ----
        # cb 0 is fp16 (precision-critical rows), cb 1..3 fp8.
        xt_tiles = {}

        def xt_dma(cb, engs=(None,)):
            xt_tiles[cb] = xt_pool.tile([128, NKC, 512], F8, tag="xt",
                                        name=f"xt_c{cb}")
            for k in range(NKC):
                eng = engs[k % len(engs)] or nc.sync
                eng.dma_start(
                    xt_tiles[cb][:, k, :],
                    xT8_d[k * 128:(k + 1) * 128,
                          (cb - 1) * 512:cb * 512])

        # prologue DMAs, ordered by first use; plain contiguous chunks only.
        # first-need-first: u_q(0,0) consumes (wq16[k], xt16[k]) pairs in k
        # order; u_q(1,0) runs just after, so xt8_1 + wq8 stream in parallel
        # on other queues.
        nc.gpsimd.dma_start(bq_sb[:], bq_d[:, :])
        xt16 = xt_pool.tile([128, NKC, 512], F16, tag="xt16", name="xt_c0")
        xt_tiles[0] = xt16
        for k in range(NKC):
            nc.scalar.dma_start(wq16_sb[:, k, :], wq16_d[k * 128:(k + 1) * 128, :])
            nc.sync.dma_start(xt16[:, k, :], xT16_d[k * 128:(k + 1) * 128, :])
        xt_dma(1, engs=(nc.gpsimd,))
        for k in range(NKC):
            eng = nc.scalar if k % 2 == 0 else nc.gpsimd
            eng.dma_start(wq8_sb[:, k, :], wq8_d[k * 128:(k + 1) * 128, :])
        for k in range(NKC):
            eng = nc.sync if k % 2 == 0 else nc.scalar
            eng.dma_start(wk16_sb[:, k, :], wk16_d[k * 128:(k + 1) * 128, :])
        for k in range(NKC):
            eng = nc.gpsimd if k % 2 == 0 else nc.scalar
            eng.dma_start(wk8_sb[:, k, :], wk8_d[k * 128:(k + 1) * 128, :])
        nc.gpsimd.dma_start(mask_lo[:], mlo_d[:, :])
        nc.gpsimd.dma_start(mask_hi[:], mhi_d[:, :])
        xt_dma(2, engs=(nc.sync, nc.gpsimd))
        for k in range(NKC):
            eng = nc.gpsimd if k % 2 == 0 else nc.scalar
            eng.dma_start(wv16_sb[:, k, :], wv16_d[k * 128:(k + 1) * 128, :])
        for k in range(NKC):
            eng = nc.sync if k % 2 == 0 else nc.gpsimd
            eng.dma_start(wv8_sb[:, k, :], wv8_d[k * 128:(k + 1) * 128, :])
        for c in range(2):
            nc.sync.dma_start(wo_sb[:, c, :], wo_d[c * 128:(c + 1) * 128, :])
        # ones block of V_aug via memsets on prologue-idle engines
        nc.gpsimd.memset(v8_sb[:, 0:NJ // 2, :, 0:DK], 1.0)
        nc.vector.memset(v8_sb[:, NJ // 2:NJ, :, 0:DK], 1.0)
        nc.vector.memset(v16_sb[:, :, :, 0:DK], 1.0)

        # ---- projection units (~2-4k PE cycles each) ----
        def u_q(cb, m):
            xt = xt_tiles[cb]
            w_sb = wq16_sb if cb == 0 else wq8_sb
            qp = ps_mi.tile([128, 512], F32, tag="mi", name=f"qp{cb}{m}")
            for k in range(NKC):
                nc.tensor.matmul(
                    qp[:], w_sb[:, k, m * 128:(m + 1) * 128],
                    xt[:, k, :], start=(k == 0), stop=(k == NKC - 1),
                )
            nc.scalar.activation(
                q_sb[:, m, cb * 512:(cb + 1) * 512], qp[:],
                mybir.ActivationFunctionType.Identity,
                bias=bq_sb[:, m:m + 1],
                scale=(1.0 if cb == 0 else 1.0 / WS),
            )

        def u_k(cb, m):
            xt = xt_tiles[cb]
            w_sb = wk16_sb if cb == 0 else wk8_sb
            kp = ps_mi.tile([128, 512], F32, tag="mi", name=f"kp{cb}{m}")
            for k in range(NKC):
                nc.tensor.matmul(
                    kp[:], w_sb[:, k, m * 128:(m + 1) * 128],
                    xt[:, k, :], start=(k == 0), stop=(k == NKC - 1),
                )
            dst = k_sb[:, m, cb * 512:(cb + 1) * 512]
            if cb == 0:
                nc.vector.tensor_copy(dst, kp[:])
            else:
                nc.vector.tensor_scalar_mul(dst, kp[:], 1.0 / WS)

        def u_v(r):
            cb = r // 4
            xt = xt_tiles[cb]
            w_sb = wv16_sb if cb == 0 else wv8_sb
            vp = ps_mi.tile([128, HPC, DK], F32, tag="mi", name=f"vp{r}")
            for k in range(NKC):
                nc.tensor.matmul(
                    vp[:], xt[:, k, (r % 4) * 128:(r % 4) * 128 + 128],
                    w_sb[:, k, :], start=(k == 0), stop=(k == NKC - 1),
                )
            if cb == 0:
                nc.vector.tensor_copy(v16_sb[:, r, :, DK:2 * DK], vp[:])
                nc.scalar.copy(v8_sb[:, r, :, DK:2 * DK], vp[:])
            else:
                nc.vector.tensor_scalar_mul(
                    v8_sb[:, r, :, DK:2 * DK], vp[:], 1.0 / WS)

        # ---- attention ----
        def attn_j(hpair, pts, J):
            width = min(640, T - J * 128)
            wA = min(512, width)
            wB = width - wA
            for part in range(2):           # row-group-alternating A then B
                for h in hpair:
                    hp = slice((h % 2) * 64, (h % 2) * 64 + 64)
                    hc = h // 2
                    if part == 0:
                        pt = pt_pool.tile([128, 640], F16, tag="pt",
                                          name=f"pt_h{h}_J{J}")
                        pts[h][J] = pt
                        s = ps_s.tile([128, 640], F32, tag="s",
                                      name=f"s_h{h}_J{J}")
                        pts[h][(J, "s")] = s
                        nc.tensor.matmul(
                            s[:, 0:wA], k_sb[hp, hc, J * 128:(J + 1) * 128],
                            q_sb[hp, hc, J * 128:J * 128 + wA],
                            start=True, stop=True,
                        )
                    else:
                        s = pts[h].pop((J, "s"))
                        pt = pts[h][J]
                        if wB > 0:
                            nc.tensor.matmul(
                                s[:, 512:512 + wB],
                                k_sb[hp, hc, J * 128:(J + 1) * 128],
                                q_sb[hp, hc, J * 128 + 512:J * 128 + width],
                                start=True, stop=True,
                            )
                        nc.scalar.activation(
                            pt[:, 0:width], s[:, 0:width],
                            mybir.ActivationFunctionType.Exp, scale=0.125,
                        )
                        nc.gpsimd.tensor_mul(pt[:, 0:128], pt[:, 0:128],
                                             mask_lo[:])
                        if width == 640:
                            nc.gpsimd.tensor_mul(pt[:, 512:640],
                                                 pt[:, 512:640], mask_hi[:])

        norm_state = {}

        def attn_pv(hpair, pts, g):
            g0 = 512 * g
            v_sb = v16_sb if g == 0 else v8_sb
            pvs = {}
            for h in hpair:
                pv = ps_pv.tile([128, 512], F32, tag="pv", name=f"pv_h{h}_g{g}")
                pvs[h] = pv
                jps = []
                for Jp in range(max(0, 4 * g - 4), 4 * g + 4):
                    wJp = min(640, T - Jp * 128)
                    lo = max(Jp * 128, g0)
                    hi = min(Jp * 128 + wJp, g0 + 512)
                    if hi > lo:
                        jps.append((Jp, lo, hi))
                # start=True lazily zeroes the whole psum bank; a full-width
                # contribution must come first
                jps.sort(key=lambda t: -(t[2] - t[1]))
                assert jps[0][2] - jps[0][1] == 512
                for idx, (Jp, lo, hi) in enumerate(jps):
                    nc.tensor.matmul(
                        pv[:, lo - g0:hi - g0],
                        v_sb[:, Jp, h, :],
                        pts[h][Jp][:, lo - Jp * 128:hi - Jp * 128],
                        start=(idx == 0), stop=(idx == len(jps) - 1),
                    )
                for Jp in range(max(0, 4 * g - 4), 4 * g):
                    pts[h].pop(Jp, None)
            norm_state[(hpair, g)] = pvs

        def attn_norm(hpair, g):
            # deferred ~1 step after attn_pv.  MUST be emitted before the pv
            # pool rotates into these tiles again (the osb multiply is the
            # psum tile's last reader).
            g0 = 512 * g
            pvs = norm_state.pop((hpair, g))
            for h in hpair:
                rcp = nrm_pool.tile([64, 512], F32, tag="rcp",
                                    name=f"rcp_h{h}_g{g}")
                nc.vector.reciprocal_approx_fast(rcp[:], pvs[h][0:64, :])
                hp = slice((h % 2) * 64, (h % 2) * 64 + 64)
                if g == 0:
                    dst = osb16[hp, h // 2, 0:512]
                else:
                    dst = osb8[hp, h // 2, g0:g0 + 512]
                nc.vector.tensor_mul(dst, pvs[h][64:128, :], rcp[:])

        def o_proj(qbs, alt_pool=False):
            # runs 1-2 steps after both pairs normalized the group; spread
            # across non-group steps so the mi-ring and the psum->SBUF copy
            # engines aren't slammed at group boundaries.  At the very tail
            # the S psum pool is retired, so its 4 banks double the po ring.
            for qb in qbs:
                so = stage_pool.tile([128, 1024], F16, tag="stage",
                                     name=f"so{qb}")
                for nh in range(2):
                    if alt_pool and nh == 1:
                        po = ps_s.tile([128, 512], F32, tag="s",
                                       name=f"po{qb}_{nh}")
                    else:
                        po = ps_mi.tile([128, 512], F32, tag="mi",
                                        name=f"po{qb}_{nh}")
                    for c in range(2):
                        if qb < 4:
                            stat = osb16[:, c, qb * 128:(qb + 1) * 128]
                        else:
                            stat = osb8[:, c, qb * 128:(qb + 1) * 128]
                        nc.tensor.matmul(
                            po[:], stat,
                            wo_sb[:, c, nh * 512:(nh + 1) * 512],
                            start=(c == 0), stop=(c == 1),
                        )
                    if nh == 0:
                        nc.scalar.copy(so[:, 0:512], po[:])
                    else:
                        nc.vector.tensor_copy(so[:, 512:1024], po[:])
                eng = (nc.sync, nc.scalar, nc.gpsimd)[qb % 3]
                eng.dma_start(out_d[qb * 128:(qb + 1) * 128, :], so[:, :])

        # ---- interleaved 18-step pipeline ----
        pt01 = {0: {}, 1: {}}
        pt23 = {2: {}, 3: {}}
        P01, P23 = (0, 1), (2, 3)
        PV01 = lambda g: attn_pv(P01, pt01, g)
        PV23 = lambda g: attn_pv(P23, pt23, g)
        N01 = lambda g: attn_norm(P01, g)
        N23 = lambda g: attn_norm(P23, g)
        XT3 = lambda: xt_dma(3, engs=(nc.sync,))
        OP = lambda qbs: o_proj(qbs)

        u_q(0, 0); u_k(0, 0); u_q(1, 0); u_k(1, 0)

        FILL = {
            0:  ([lambda: u_q(0, 1)],  [lambda: u_k(0, 1)]),
            1:  ([lambda: u_q(1, 1)],  [lambda: u_q(2, 0), lambda: u_v(0)]),
            2:  ([lambda: u_v(1)],     [lambda: u_v(2), lambda: u_v(3)]),
            3:  ([lambda: u_q(2, 1)],  [lambda: PV01(0), XT3]),
            4:  ([lambda: u_k(1, 1)],  [lambda: N01(0), lambda: PV23(0), lambda: u_v(4)]),
            5:  ([lambda: u_k(2, 0)],  [lambda: N23(0), lambda: OP([0, 1]), lambda: u_v(5)]),
            6:  ([lambda: u_k(2, 1)],  [lambda: OP([2, 3]), lambda: u_v(6)]),
            7:  ([lambda: u_v(7)],     [lambda: PV01(1), lambda: u_q(3, 0)]),
            8:  ([lambda: u_q(3, 1)],  [lambda: N01(1), lambda: PV23(1), lambda: u_v(8)]),
            9:  ([lambda: u_v(9)],     [lambda: N23(1), lambda: OP([4, 5])]),
            10: ([lambda: u_k(3, 0)],  [lambda: OP([6, 7]), lambda: u_v(10)]),
            11: ([lambda: u_v(11)],    [lambda: PV01(2)]),
            12: ([lambda: u_k(3, 1)],  [lambda: N01(2), lambda: PV23(2), lambda: u_v(12)]),
            13: ([lambda: u_v(13)],    [lambda: N23(2), lambda: OP([8, 9])]),
            14: ([lambda: u_v(14)],    [lambda: OP([10, 11]), lambda: u_v(15)]),
        }

        for s in range(15):
            pre, post = FILL[s]
            if s < NJ:
                attn_j(P01, pt01, s)
            for f in pre:
                f()
            if s >= 1:
                attn_j(P23, pt23, s - 1)
            if s == 14:
                attn_j(P23, pt23, 14)
            for f in post:
                f()
        # compressed tail: both pairs' last S, PV, norms and the final
        # output projections in one step.  ACT is idle here (no exps left),
        # so it takes the denominator copies off the DVE critical path.
        attn_j(P01, pt01, 15)
        attn_j(P23, pt23, 15)
        attn_pv(P01, pt01, 3)
        attn_norm(P01, 3)
        attn_pv(P23, pt23, 3)
        attn_norm(P23, 3)
        o_proj([12, 13, 14, 15], alt_pool=True)


def _build():
    if "nc" in _NC_CACHE:
        return _NC_CACHE["nc"]
    nc = bacc.Bacc("TRN2", debug=False)
    with tile.TileContext(nc) as tc:
        _emit(tc)
    nc.compile()
    _NC_CACHE["nc"] = nc
    return nc


def _shard_inputs(x, Wq, bq, Wk, Wv, Wo):
    import ml_dtypes
    NP8 = ml_dtypes.float8_e4m3fn
    idx = np.arange(128)
    mlo = (idx[None, :] >= idx[:, None]).astype(np.float16)  # c >= p
    mhi = (idx[None, :] < idx[:, None]).astype(np.float16)   # c < p
    in_maps = []
    for b in range(2):
        xT = np.ascontiguousarray(x[b].T)
        xT16 = xT[:, 0:512].astype(np.float16)
        xT8 = xT[:, 512:].astype(NP8)
        for hg in range(4):
            cols = slice(hg * HCOLS, (hg + 1) * HCOLS)
            in_maps.append({
                "xT16": xT16,
                "xT8": xT8,
                "wq16": np.ascontiguousarray(Wq[:, cols]).astype(np.float16),
                "wk16": np.ascontiguousarray(Wk[:, cols]).astype(np.float16),
                "wv16": np.ascontiguousarray(Wv[:, cols]).astype(np.float16),
                "wq8": np.ascontiguousarray(Wq[:, cols] * WS).astype(NP8),
                "wk8": np.ascontiguousarray(Wk[:, cols] * WS).astype(NP8),
                "wv8": np.ascontiguousarray(Wv[:, cols] * WS).astype(NP8),
                "wo": np.ascontiguousarray(Wo[cols, :]).astype(np.float16),
                "bqp": np.ascontiguousarray(bq[cols].reshape(2, 128).T),
                "mlo": mlo, "mhi": mhi,
            })
    return in_maps


def kernel(x, Wq, bq, Wk, bk, Wv, bv, Wo, bo, _trace=False, _tmpdir=None):
    x = np.asarray(x, dtype=np.float32)
    Wq = np.asarray(Wq, dtype=np.float32)
    Wk = np.asarray(Wk, dtype=np.float32)
    Wv = np.asarray(Wv, dtype=np.float32)
    Wo = np.asarray(Wo, dtype=np.float32)
    bq = np.asarray(bq, dtype=np.float32)
    bv = np.asarray(bv, dtype=np.float32)
    bo = np.asarray(bo, dtype=np.float32)

    nc = _build()
    in_maps = _shard_inputs(x, Wq, bq, Wk, Wv, Wo)
    res = run_bass_kernel_spmd(
        nc, in_maps, core_ids=list(range(8)), trace=_trace, tmpdir=_tmpdir,
    )
    host_bias = (bv @ Wo + bo).astype(np.float32)
    out = np.zeros((2, T, D), dtype=np.float32)
    for b in range(2):
        acc = res.results[b * 4]["out"].astype(np.float32).copy()
        for hg in range(1, 4):
            acc += res.results[b * 4 + hg]["out"]
        out[b] = acc + host_bias
    kernel._last_results = res
    return out


# revision 9
# speedup vs baseline: 1.0151x; 1.0151x over previous
"""Sliding-window causal self-attention (B=2, T=2048, D=1024, H=16, dk=64, W=512)
on 8 Trainium2 NeuronCores.

Sharding: core = (b, hg) for b in {0,1}, head-group hg in {0..3}.
Data parallel over batch, tensor parallel over heads: each core gets
x[b]^T, the 4-head column slices of Wq/Wk/Wv (+bq slice) and the matching
row slice of Wo, and produces a partial [T, D] output (fp16).  Host gathers
with out[b] = sum_hg partial[b,hg] + (bv @ Wo + bo) in fp32.

Math notes (exact softmax identities, validated vs reference):
 - bk shifts every logit of a row by a per-row constant -> cancels in softmax.
 - bv enters the output linearly with weights summing to 1 -> folded into the
   host-side bias term bv @ Wo (+ bo), added once after the cross-core sum.
 - no max-subtraction in softmax: logits are O(1), fp16 exp is safe
   (|S/8| < 6 -> exp < 403 << 65504).

Precision plan (v4): measured on HW, a rotating fp16 [128,128] stationary
exposes ~40-210ns of LDWEIGHTS per matmul, while fp8 stationaries (and
64-row fp16 stationaries) hide it completely.  fp8 also halves the input
DMA.  But fp8 fails accuracy for output rows < 512 (tiny attention windows
concentrate quantization error: row r averages ~0.37*r keys, so early rows
see individual-V-element error unaveraged).  Hybrid:
 - rows/keys 0..511 ("g0"): x, W stationaries, V, osb all fp16.
 - rows 512+: x (fp8 moving+stationary), Wq/Wk/Wv (x32-prescaled fp8
   stationaries, descaled in the psum->SBUF copy), V_aug fp8 stationary,
   osb fp8 stationary.  Q, K, pt (exp), Wo moving stay fp16 everywhere —
   moving dtype doesn't change the PE stream rate, so fp16 there is free
   accuracy.  Numpy-simulated end-to-end rel err 8.9e-3 (gate 2e-2).
S stationaries (K) are 64-row fp16 -> already hidden; left fp16.

The PE p-state ramp costs ~2x for the first ~3.5us of continuous matmul
work, and the first exp pays a ~1.3us activation-table load; both are
prepaid during the DMA head with warmup matmuls / a dummy exp on a
memset tile.

Schedule (v3): both head-pairs' attention J-loops run interleaved in one
18-step pipeline (pair23 lags pair01 by 2 steps), with the Q/K/V projection
work chopped into ~2-4k-cycle units and woven between the S-matmul blocks so
the PE never starves while the ACT (exp) pipeline drains.

Step s: S(pair01, J=s) | pre-filler | S(pair23, J=s-2) | post-filler+groups.
PV groups: pair01 at s=4g+3, pair23 at s=4g+5; the output projection for
group g runs right after pair23's group g normalizes (all 4 heads ready).

The V_aug stationary carries a 64-wide ones block ahead of the 64 V
columns, so the PV matmul emits the softmax denominator already broadcast
across psum partitions 0:64 -- no denominator copy and no rank-1 broadcast
matmul (and the custom-DVE reciprocal reads psum at base partition 0; it
returns garbage on hardware at base 64).  Input DMAs are plain contiguous
per-chunk transfers (a rearranged multi-descriptor DMA's completion
semaphore was observed to fire before all bytes landed, corrupting
first-run results).
"""

import math
from contextlib import ExitStack

import numpy as np

import concourse.bass as bass
import concourse.mybir as mybir
import concourse.tile as tile
from concourse import bacc
from concourse.bass_utils import run_bass_kernel_spmd

F32 = mybir.dt.float32
F16 = mybir.dt.float16
F8 = mybir.dt.float8e4

T = 2048
D = 1024
NHEAD = 16
DK = 64
WINDOW = 512
HPC = 4            # heads per core
HCOLS = HPC * DK   # 256 projected columns per core
NJ = T // 128      # 16 j/query blocks
NKC = D // 128     # 8 contraction chunks over D
NG = 4             # query-block groups of 512
WS = 32.0          # fp8 weight prescale (W sigma = 1/32 -> sigma 1)

_NC_CACHE = {}


def _emit(tc):
    nc = tc.nc
    xT16_d = nc.dram_tensor("xT16", [D, 512], F16, kind="ExternalInput").ap()
    xT8_d = nc.dram_tensor("xT8", [D, T - 512], F8, kind="ExternalInput").ap()
    wq16_d = nc.dram_tensor("wq16", [D, HCOLS], F16, kind="ExternalInput").ap()
    wk16_d = nc.dram_tensor("wk16", [D, HCOLS], F16, kind="ExternalInput").ap()
    wv16_d = nc.dram_tensor("wv16", [D, HCOLS], F16, kind="ExternalInput").ap()
    wq8_d = nc.dram_tensor("wq8", [D, HCOLS], F8, kind="ExternalInput").ap()
    wk8_d = nc.dram_tensor("wk8", [D, HCOLS], F8, kind="ExternalInput").ap()
    wv8_d = nc.dram_tensor("wv8", [D, HCOLS], F8, kind="ExternalInput").ap()
    wo_d = nc.dram_tensor("wo", [HCOLS, D], F16, kind="ExternalInput").ap()
    bq_d = nc.dram_tensor("bqp", [128, 2], F32, kind="ExternalInput").ap()
    mlo_d = nc.dram_tensor("mlo", [128, 128], F16, kind="ExternalInput").ap()
    mhi_d = nc.dram_tensor("mhi", [128, 128], F16, kind="ExternalInput").ap()
    out_d = nc.dram_tensor("out", [T, D], F16, kind="ExternalOutput").ap()

    with ExitStack() as ctx:
        const_pool = ctx.enter_context(tc.tile_pool(name="const", bufs=1))
        qk_pool = ctx.enter_context(tc.tile_pool(name="qk", bufs=1))
        w_pool = ctx.enter_context(tc.tile_pool(name="w", bufs=1))
        xt_pool = ctx.enter_context(tc.tile_pool(name="xt", bufs=3))
        pt_pool = ctx.enter_context(tc.tile_pool(name="pt", bufs=36))
        nrm_pool = ctx.enter_context(tc.tile_pool(name="nrm", bufs=4))
        stage_pool = ctx.enter_context(tc.tile_pool(name="stage", bufs=2))
        ps_s = ctx.enter_context(tc.tile_pool(name="ps_s", bufs=2, space="PSUM"))
        ps_pv = ctx.enter_context(tc.tile_pool(name="ps_pv", bufs=2, space="PSUM"))
        ps_mi = ctx.enter_context(tc.tile_pool(name="ps_mi", bufs=2, space="PSUM"))

        bq_sb = const_pool.tile([128, 2], F32)
        mask_lo = const_pool.tile([128, 128], F16)   # keep c >= p (upper incl)
        mask_hi = const_pool.tile([128, 128], F16)   # keep c < p (strict lower)
        warm = const_pool.tile([128, 640], F16)
        expw = const_pool.tile([128, 16], F16)

        wo_sb = qk_pool.tile([128, 2, D], F16)
        # V storage [j-part, J, head, 2*dk]; cols 0:64 of each head slot
        # are 1.0, so the PV matmul emits the softmax denominator already
        # broadcast across psum partitions 0:64.  v8 covers all J (fp8
        # stationary = hidden LDWEIGHTS); v16 duplicates J0..3 for the
        # precision-critical g0 PV.
        v8_sb = qk_pool.tile([128, NJ, HPC, 2 * DK], F8)
        v16_sb = qk_pool.tile([128, 4, HPC, 2 * DK], F16)
        q_sb = qk_pool.tile([128, 2, T], F16)
        k_sb = qk_pool.tile([128, 2, T], F16)
        osb16 = qk_pool.tile([128, 2, 512], F16)   # normalized O^T, g0
        osb8 = qk_pool.tile([128, 2, T], F8)       # normalized O^T, g1..3

        wq16_sb = w_pool.tile([128, NKC, HCOLS], F16)
        wk16_sb = w_pool.tile([128, NKC, HCOLS], F16)
        wv16_sb = w_pool.tile([128, NKC, HCOLS], F16)
        wq8_sb = w_pool.tile([128, NKC, HCOLS], F8)
        wk8_sb = w_pool.tile([128, NKC, HCOLS], F8)
        wv8_sb = w_pool.tile([128, NKC, HCOLS], F8)

        # ---- PE p-state + exp-table warmup (runs inside the DMA head) ----
        nc.vector.memset(warm[:], 0.0)
        nc.scalar.activation(expw[:], warm[:, 0:16],
                             mybir.ActivationFunctionType.Exp, scale=0.125)
        wtile = ps_mi.tile([128, 512], F32, tag="mi", name="warm")
        for _ in range(6):
            nc.tensor.matmul(wtile[:], warm[:, 0:128], warm[:, 128:640],
                             start=True, stop=True)

        # ---- x^T streamed by 512-column blocks ----
        # cb 0 is fp16 (precision-critical rows), cb 1..3 fp8.
        xt_tiles = {}

        def xt_dma(cb, engs=(None,)):
            xt_tiles[cb] = xt_pool.tile([128, NKC, 512], F8, tag="xt",
                                        name=f"xt_c{cb}")
            for k in range(NKC):
                eng = engs[k % len(engs)] or nc.sync
                eng.dma_start(
                    xt_tiles[cb][:, k, :],
                    xT8_d[k * 128:(k + 1) * 128,
                          (cb - 1) * 512:cb * 512])

        # prologue DMAs in strict first-use order, spread across the three
        # DMA-capable engines (sync/scalar/gpsimd); descriptors fan out over
        # all 16 hw rings so global issue order ~= service order.  Plain
        # contiguous chunks only.
        # Phase 1 (needed from t~3us): u_q/u_k(0,*) inputs, k-ordered.
        nc.gpsimd.dma_start(bq_sb[:], bq_d[:, :])
        xt16 = xt_pool.tile([128, NKC, 512], F16, tag="xt16", name="xt_c0")
        xt_tiles[0] = xt16
        for k in range(NKC):
            nc.sync.dma_start(xt16[:, k, :], xT16_d[k * 128:(k + 1) * 128, :])
            nc.scalar.dma_start(wq16_sb[:, k, :], wq16_d[k * 128:(k + 1) * 128, :])
            nc.gpsimd.dma_start(wk16_sb[:, k, :], wk16_d[k * 128:(k + 1) * 128, :])
        # Phase 2 (t~10us): masks for the first exp, cb1 inputs.
        nc.gpsimd.dma_start(mask_lo[:], mlo_d[:, :])
        nc.gpsimd.dma_start(mask_hi[:], mhi_d[:, :])
        xt_tiles[1] = xt_pool.tile([128, NKC, 512], F8, tag="xt", name="xt_c1")
        for k in range(NKC):
            nc.sync.dma_start(
                xt_tiles[1][:, k, :], xT8_d[k * 128:(k + 1) * 128, 0:512])
            nc.scalar.dma_start(wq8_sb[:, k, :], wq8_d[k * 128:(k + 1) * 128, :])
            nc.gpsimd.dma_start(wk8_sb[:, k, :], wk8_d[k * 128:(k + 1) * 128, :])
        # Phase 3 (t~12us+): wv16 for u_v(0..3), cb2, then the fp8 V weights.
        for k in range(NKC):
            eng = nc.scalar if k % 2 == 0 else nc.gpsimd
            eng.dma_start(wv16_sb[:, k, :], wv16_d[k * 128:(k + 1) * 128, :])
        xt_dma(2, engs=(nc.sync,))
        for k in range(NKC):
            eng = nc.scalar if k % 2 == 0 else nc.gpsimd
            eng.dma_start(wv8_sb[:, k, :], wv8_d[k * 128:(k + 1) * 128, :])
        for c in range(2):
            nc.sync.dma_start(wo_sb[:, c, :], wo_d[c * 128:(c + 1) * 128, :])
        # ones block of V_aug via memsets on prologue-idle engines
        nc.gpsimd.memset(v8_sb[:, 0:NJ // 2, :, 0:DK], 1.0)
        nc.vector.memset(v8_sb[:, NJ // 2:NJ, :, 0:DK], 1.0)
        nc.vector.memset(v16_sb[:, :, :, 0:DK], 1.0)

        # ---- projection units (~2-4k PE cycles each) ----
        def u_q(cb, m):
            xt = xt_tiles[cb]
            w_sb = wq16_sb if cb == 0 else wq8_sb
            qp = ps_mi.tile([128, 512], F32, tag="mi", name=f"qp{cb}{m}")
            for k in range(NKC):
                nc.tensor.matmul(
                    qp[:], w_sb[:, k, m * 128:(m + 1) * 128],
                    xt[:, k, :], start=(k == 0), stop=(k == NKC - 1),
                )
            nc.scalar.activation(
                q_sb[:, m, cb * 512:(cb + 1) * 512], qp[:],
                mybir.ActivationFunctionType.Identity,
                bias=bq_sb[:, m:m + 1],
                scale=(1.0 if cb == 0 else 1.0 / WS),
            )

        def u_k(cb, m):
            xt = xt_tiles[cb]
            w_sb = wk16_sb if cb == 0 else wk8_sb
            kp = ps_mi.tile([128, 512], F32, tag="mi", name=f"kp{cb}{m}")
            for k in range(NKC):
                nc.tensor.matmul(
                    kp[:], w_sb[:, k, m * 128:(m + 1) * 128],
                    xt[:, k, :], start=(k == 0), stop=(k == NKC - 1),
                )
            dst = k_sb[:, m, cb * 512:(cb + 1) * 512]
            if cb == 0:
                nc.vector.tensor_copy(dst, kp[:])
            else:
                nc.vector.tensor_scalar_mul(dst, kp[:], 1.0 / WS)

        def u_v(r):
            cb = r // 4
            xt = xt_tiles[cb]
            w_sb = wv16_sb if cb == 0 else wv8_sb
            vp = ps_mi.tile([128, HPC, DK], F32, tag="mi", name=f"vp{r}")
            for k in range(NKC):
                nc.tensor.matmul(
                    vp[:], xt[:, k, (r % 4) * 128:(r % 4) * 128 + 128],
                    w_sb[:, k, :], start=(k == 0), stop=(k == NKC - 1),
                )
            if cb == 0:
                nc.vector.tensor_copy(v16_sb[:, r, :, DK:2 * DK], vp[:])
                nc.scalar.copy(v8_sb[:, r, :, DK:2 * DK], vp[:])
            else:
                nc.vector.tensor_scalar_mul(
                    v8_sb[:, r, :, DK:2 * DK], vp[:], 1.0 / WS)

        # ---- attention ----
        # split into A (the wA S-matmul; allocates the psum + pt tiles) and
        # B (wB S-matmul + exp + masks) so the schedule can run A(J) a full
        # step before B(J): the exp drains while unrelated PE work streams,
        # and the J=0 A-part only needs cb0 (fp16) inputs while the fp8 x
        # chunks are still in flight.
        def attn_a(hpair, pts, J):
            wA = min(512, T - J * 128)
            for h in hpair:
                hp = slice((h % 2) * 64, (h % 2) * 64 + 64)
                hc = h // 2
                pt = pt_pool.tile([128, 640], F16, tag="pt",
                                  name=f"pt_h{h}_J{J}")
                pts[h][J] = pt
                s = ps_s.tile([128, 640], F32, tag="s", name=f"s_h{h}_J{J}")
                pts[h][(J, "s")] = s
                nc.tensor.matmul(
                    s[:, 0:wA], k_sb[hp, hc, J * 128:(J + 1) * 128],
                    q_sb[hp, hc, J * 128:J * 128 + wA],
                    start=True, stop=True,
                )

        def attn_b(hpair, pts, J):
            width = min(640, T - J * 128)
            wB = width - min(512, width)
            for h in hpair:
                hp = slice((h % 2) * 64, (h % 2) * 64 + 64)
                hc = h // 2
                s = pts[h].pop((J, "s"))
                pt = pts[h][J]
                if wB > 0:
                    nc.tensor.matmul(
                        s[:, 512:512 + wB],
                        k_sb[hp, hc, J * 128:(J + 1) * 128],
                        q_sb[hp, hc, J * 128 + 512:J * 128 + width],
                        start=True, stop=True,
                    )
                nc.scalar.activation(
                    pt[:, 0:width], s[:, 0:width],
                    mybir.ActivationFunctionType.Exp, scale=0.125,
                )
                nc.gpsimd.tensor_mul(pt[:, 0:128], pt[:, 0:128], mask_lo[:])
                if width == 640:
                    nc.gpsimd.tensor_mul(pt[:, 512:640], pt[:, 512:640],
                                         mask_hi[:])

        norm_state = {}

        def _pv_ranges(g, half):
            # column range, contributing J range, and pt tiles retired after.
            if half is None:
                return (512 * g, 512 * g + 512,
                        max(0, 4 * g - 4), 4 * g + 3,
                        range(max(0, 4 * g - 4), 4 * g))
            if half == "a":
                return (512 * g, 512 * g + 256,
                        4 * g - 4, 4 * g + 1,
                        range(4 * g - 4, 4 * g - 2))
            return (512 * g + 256, 512 * g + 512,
                    4 * g - 2, 4 * g + 3,
                    range(4 * g - 2, 4 * g + 4))

        def attn_pv(hpair, pts, g, half=None):
            collo, colhi, jlo, jhi, _ = _pv_ranges(g, half)
            v_sb = v16_sb if g == 0 else v8_sb
            pvs = {}
            for h in hpair:
                pv = ps_pv.tile([128, colhi - collo], F32, tag="pv",
                                name=f"pv_h{h}_g{g}{half or ''}")
                pvs[h] = pv
                jps = []
                for Jp in range(jlo, jhi + 1):
                    wJp = min(640, T - Jp * 128)
                    lo = max(Jp * 128, collo)
                    hi = min(Jp * 128 + wJp, colhi)
                    if hi > lo:
                        jps.append((Jp, lo, hi))
                # start=True lazily zeroes the whole psum bank; a full-width
                # contribution must come first
                jps.sort(key=lambda t: -(t[2] - t[1]))
                assert jps[0][2] - jps[0][1] == colhi - collo
                for idx, (Jp, lo, hi) in enumerate(jps):
                    nc.tensor.matmul(
                        pv[:, lo - collo:hi - collo],
                        v_sb[:, Jp, h, :],
                        pts[h][Jp][:, lo - Jp * 128:hi - Jp * 128],
                        start=(idx == 0), stop=(idx == len(jps) - 1),
                    )
            norm_state[(hpair, g, half)] = pvs
            for h in hpair:
                for Jp in _pv_ranges(g, half)[4]:
                    pts[h].pop(Jp, None)

        def attn_norm(hpair, g, half=None):
            # deferred ~1 step after attn_pv.  MUST be emitted before the pv
            # pool rotates into these tiles again (the osb multiply is the
            # psum tile's last reader).
            collo, colhi, _, _, _ = _pv_ranges(g, half)
            pvs = norm_state.pop((hpair, g, half))
            for h in hpair:
                rcp = nrm_pool.tile([64, colhi - collo], F32, tag="rcp",
                                    name=f"rcp_h{h}_g{g}{half or ''}")
                nc.vector.reciprocal_approx_fast(rcp[:], pvs[h][0:64, :])
                hp = slice((h % 2) * 64, (h % 2) * 64 + 64)
                if g == 0:
                    dst = osb16[hp, h // 2, 0:512]
                else:
                    dst = osb8[hp, h // 2, collo:colhi]
                nc.vector.tensor_mul(dst, pvs[h][64:128, :], rcp[:])

        def o_proj(qbs, alt_pool=False):
            # runs 1-2 steps after both pairs normalized the group; spread
            # across non-group steps so the mi-ring and the psum->SBUF copy
            # engines aren't slammed at group boundaries.  At the very tail
            # the S psum pool is retired, so its 4 banks double the po ring.
            for qb in qbs:
                so = stage_pool.tile([128, 1024], F16, tag="stage",
                                     name=f"so{qb}")
                for nh in range(2):
                    if alt_pool and nh == 1:
                        po = ps_s.tile([128, 512], F32, tag="s",
                                       name=f"po{qb}_{nh}")
                    else:
                        po = ps_mi.tile([128, 512], F32, tag="mi",
                                        name=f"po{qb}_{nh}")
                    for c in range(2):
                        if qb < 4:
                            stat = osb16[:, c, qb * 128:(qb + 1) * 128]
                        else:
                            stat = osb8[:, c, qb * 128:(qb + 1) * 128]
                        nc.tensor.matmul(
                            po[:], stat,
                            wo_sb[:, c, nh * 512:(nh + 1) * 512],
                            start=(c == 0), stop=(c == 1),
                        )
                    if nh == 0:
                        nc.scalar.copy(so[:, 0:512], po[:])
                    else:
                        nc.vector.tensor_copy(so[:, 512:1024], po[:])
                eng = (nc.sync, nc.scalar, nc.gpsimd)[qb % 3]
                eng.dma_start(out_d[qb * 128:(qb + 1) * 128, :], so[:, :])

        # ---- interleaved 19-step pipeline ----
        # Step s runs [B01(s-1), A01(s), B23(s-2), A23(s-1)] + fillers.
        # pair01's J=0 A-part needs only cb0 (fp16) data, so the PE starts
        # while the fp8 x chunks are still streaming; pair23 lags pair01 by
        # one J so the s-psum ring (2 bufs) alternates cleanly.  Group-3
        # PV/norm/o_proj are split in column halves (a: cols 1536:1792 from
        # J8..13, b: cols 1792:2048 from J10..15) so OP(12,13) overlaps the
        # final S blocks instead of serializing after them.
        pt01 = {0: {}, 1: {}}
        pt23 = {2: {}, 3: {}}
        P01, P23 = (0, 1), (2, 3)
        A01 = lambda J: attn_a(P01, pt01, J)
        B01 = lambda J: attn_b(P01, pt01, J)
        A23 = lambda J: attn_a(P23, pt23, J)
        B23 = lambda J: attn_b(P23, pt23, J)
        PV01 = lambda g, hf=None: attn_pv(P01, pt01, g, hf)
        PV23 = lambda g, hf=None: attn_pv(P23, pt23, g, hf)
        N01 = lambda g, hf=None: attn_norm(P01, g, hf)
        N23 = lambda g, hf=None: attn_norm(P23, g, hf)
        XT3 = lambda: xt_dma(3, engs=(nc.sync,))
        OP = lambda qbs, alt=False: o_proj(qbs, alt_pool=alt)

        u_q(0, 0)
        u_k(0, 0)

        # Fillers must be emitted no later than the step BEFORE the core op
        # that consumes them (PE executes in emission order): u_k(c,1) before
        # A23(4c) at s4c+1, u_q(3,*) before B01(8)/A23(9) at s9/s10, etc.
        FILL = {
            0:  [lambda: u_q(0, 1), lambda: u_k(0, 1), lambda: u_q(1, 0)],
            1:  [lambda: u_q(1, 1), lambda: u_v(0), lambda: u_v(1)],
            2:  [lambda: u_v(2), lambda: u_v(3), lambda: u_k(1, 0)],
            3:  [lambda: u_q(2, 0), XT3],
            4:  [lambda: PV01(0), lambda: u_k(1, 1)],
            5:  [lambda: N01(0), lambda: u_q(2, 1), lambda: u_v(4)],
            6:  [lambda: PV23(0), lambda: u_k(2, 0), lambda: u_v(5)],
            7:  [lambda: N23(0), lambda: OP([0, 1]), lambda: u_v(6),
                 lambda: u_q(3, 0)],
            8:  [lambda: u_v(7), lambda: PV01(1), lambda: u_k(2, 1)],
            9:  [lambda: N01(1), lambda: OP([2, 3]), lambda: u_v(8),
                 lambda: u_q(3, 1)],
            10: [lambda: PV23(1), lambda: u_v(9)],
            11: [lambda: N23(1), lambda: OP([4, 5]), lambda: u_k(3, 0),
                 lambda: u_v(10)],
            12: [lambda: u_v(11), lambda: PV01(2), lambda: u_k(3, 1)],
            13: [lambda: N01(2), lambda: OP([6, 7]), lambda: u_v(12)],
            14: [lambda: u_v(13), lambda: PV23(2), lambda: PV01(3, "a")],
            15: [lambda: N23(2), lambda: N01(3, "a"), lambda: OP([8, 9])],
            16: [lambda: u_v(14), lambda: u_v(15), lambda: PV23(3, "a"),
                 lambda: N23(3, "a"), lambda: OP([10, 11])],
            17: [lambda: OP([12, 13]), lambda: PV01(3, "b"),
                 lambda: N01(3, "b")],
            18: [lambda: PV23(3, "b"), lambda: N23(3, "b"),
                 lambda: OP([14, 15], True)],
        }

        for s in range(19):
            if 1 <= s <= 16:
                B01(s - 1)
            if s <= 15:
                A01(s)
            if 2 <= s <= 17:
                B23(s - 2)
            if 1 <= s <= 16:
                A23(s - 1)
            for f in FILL[s]:
                f()


def _build():
    if "nc" in _NC_CACHE:
        return _NC_CACHE["nc"]
    nc = bacc.Bacc("TRN2", debug=False)
    with tile.TileContext(nc) as tc:
        _emit(tc)
    nc.compile()
    _NC_CACHE["nc"] = nc
    return nc


def _shard_inputs(x, Wq, bq, Wk, Wv, Wo):
    import ml_dtypes
    NP8 = ml_dtypes.float8_e4m3fn
    idx = np.arange(128)
    mlo = (idx[None, :] >= idx[:, None]).astype(np.float16)  # c >= p
    mhi = (idx[None, :] < idx[:, None]).astype(np.float16)   # c < p
    in_maps = []
    for b in range(2):
        xT = np.ascontiguousarray(x[b].T)
        xT16 = xT[:, 0:512].astype(np.float16)
        xT8 = xT[:, 512:].astype(NP8)
        for hg in range(4):
            cols = slice(hg * HCOLS, (hg + 1) * HCOLS)
            in_maps.append({
                "xT16": xT16,
                "xT8": xT8,
                "wq16": np.ascontiguousarray(Wq[:, cols]).astype(np.float16),
                "wk16": np.ascontiguousarray(Wk[:, cols]).astype(np.float16),
                "wv16": np.ascontiguousarray(Wv[:, cols]).astype(np.float16),
                "wq8": np.ascontiguousarray(Wq[:, cols] * WS).astype(NP8),
                "wk8": np.ascontiguousarray(Wk[:, cols] * WS).astype(NP8),
                "wv8": np.ascontiguousarray(Wv[:, cols] * WS).astype(NP8),
                "wo": np.ascontiguousarray(Wo[cols, :]).astype(np.float16),
                "bqp": np.ascontiguousarray(bq[cols].reshape(2, 128).T),
                "mlo": mlo, "mhi": mhi,
            })
    return in_maps


def kernel(x, Wq, bq, Wk, bk, Wv, bv, Wo, bo, _trace=False, _tmpdir=None):
    x = np.asarray(x, dtype=np.float32)
    Wq = np.asarray(Wq, dtype=np.float32)
    Wk = np.asarray(Wk, dtype=np.float32)
    Wv = np.asarray(Wv, dtype=np.float32)
    Wo = np.asarray(Wo, dtype=np.float32)
    bq = np.asarray(bq, dtype=np.float32)
    bv = np.asarray(bv, dtype=np.float32)
    bo = np.asarray(bo, dtype=np.float32)

    nc = _build()
    in_maps = _shard_inputs(x, Wq, bq, Wk, Wv, Wo)
    res = run_bass_kernel_spmd(
        nc, in_maps, core_ids=list(range(8)), trace=_trace, tmpdir=_tmpdir,
    )
    host_bias = (bv @ Wo + bo).astype(np.float32)
    out = np.zeros((2, T, D), dtype=np.float32)
    for b in range(2):
        acc = res.results[b * 4]["out"].astype(np.float32).copy()
        for hg in range(1, 4):
            acc += res.results[b * 4 + hg]["out"]
        out[b] = acc + host_bias
    kernel._last_results = res
    return out


# revision 12
# speedup vs baseline: 1.0960x; 1.0797x over previous
"""Sliding-window causal self-attention (B=2, T=2048, D=1024, H=16, dk=64, W=512)
on 8 Trainium2 NeuronCores.

Sharding: core = (b, hg) for b in {0,1}, head-group hg in {0..3}.
Data parallel over batch, tensor parallel over heads: each core gets
x[b]^T, the 4-head column slices of Wq/Wk/Wv (+bq slice) and the matching
row slice of Wo, and produces a partial [T, D] output (fp16).  Host gathers
with out[b] = sum_hg partial[b,hg] + (bv @ Wo + bo) in fp32.

Math notes (exact softmax identities, validated vs reference):
 - bk shifts every logit of a row by a per-row constant -> cancels in softmax.
 - bv enters the output linearly with weights summing to 1 -> folded into the
   host-side bias term bv @ Wo (+ bo), added once after the cross-core sum.
 - no max-subtraction in softmax: logits are O(1), fp16 exp is safe
   (|S/8| < 6 -> exp < 403 << 65504).

Precision plan (v4): measured on HW, a rotating fp16 [128,128] stationary
exposes ~40-210ns of LDWEIGHTS per matmul, while fp8 stationaries (and
64-row fp16 stationaries) hide it completely.  fp8 also halves the input
DMA.  But fp8 fails accuracy for output rows < 512 (tiny attention windows
concentrate quantization error: row r averages ~0.37*r keys, so early rows
see individual-V-element error unaveraged).  Hybrid:
 - rows/keys 0..511 ("g0"): x, W stationaries, V, osb all fp16.
 - rows 512+: x (fp8 moving+stationary), Wq/Wk/Wv (x32-prescaled fp8
   stationaries, descaled in the psum->SBUF copy), V_aug fp8 stationary,
   osb fp8 stationary.  Q, K, pt (exp), Wo moving stay fp16 everywhere —
   moving dtype doesn't change the PE stream rate, so fp16 there is free
   accuracy.  Numpy-simulated end-to-end rel err 8.9e-3 (gate 2e-2).
S stationaries (K) are 64-row fp16 -> already hidden; left fp16.

The PE p-state ramp costs ~2x for the first ~3.5us of continuous matmul
work, and the first exp pays a ~1.3us activation-table load; both are
prepaid during the DMA head with warmup matmuls / a dummy exp on a
memset tile.

Schedule (v3): both head-pairs' attention J-loops run interleaved in one
18-step pipeline (pair23 lags pair01 by 2 steps), with the Q/K/V projection
work chopped into ~2-4k-cycle units and woven between the S-matmul blocks so
the PE never starves while the ACT (exp) pipeline drains.

Step s: S(pair01, J=s) | pre-filler | S(pair23, J=s-2) | post-filler+groups.
PV groups: pair01 at s=4g+3, pair23 at s=4g+5; the output projection for
group g runs right after pair23's group g normalizes (all 4 heads ready).

The V_aug stationary carries a 64-wide ones block ahead of the 64 V
columns, so the PV matmul emits the softmax denominator already broadcast
across psum partitions 0:64 -- no denominator copy and no rank-1 broadcast
matmul (and the custom-DVE reciprocal reads psum at base partition 0; it
returns garbage on hardware at base 64).  Input DMAs are plain contiguous
per-chunk transfers (a rearranged multi-descriptor DMA's completion
semaphore was observed to fire before all bytes landed, corrupting
first-run results).
"""

import math
from contextlib import ExitStack

import numpy as np

import concourse.bass as bass
import concourse.mybir as mybir
import concourse.tile as tile
from concourse import bacc
from concourse.bass_utils import run_bass_kernel_spmd

F32 = mybir.dt.float32
F16 = mybir.dt.float16
F8 = mybir.dt.float8e4

T = 2048
D = 1024
NHEAD = 16
DK = 64
WINDOW = 512
HPC = 4            # heads per core
HCOLS = HPC * DK   # 256 projected columns per core
NJ = T // 128      # 16 j/query blocks
NKC = D // 128     # 8 contraction chunks over D
NG = 4             # query-block groups of 512
WS = 32.0          # fp8 weight prescale (W sigma = 1/32 -> sigma 1)

_NC_CACHE = {}


def _emit(tc):
    nc = tc.nc
    # All bulk inputs arrive pre-tiled by the host into SBUF partition-major
    # layout ([128, chunk, cols]), so each tensor is ONE wide dma_start whose
    # per-partition bytes are contiguous: ~16 large descriptors fan out over
    # the 16 DMA rings instead of ~86 issues x 128 small descriptors (each
    # dma_start costs ~600ns of issue time on its engine, which starved the
    # prologue).
    xT16_d = nc.dram_tensor("xT16", [128, NKC, 512], F16, kind="ExternalInput").ap()
    xT8_d = nc.dram_tensor("xT8", [128, 3, NKC, 512], F8, kind="ExternalInput").ap()
    wq16_d = nc.dram_tensor("wq16", [128, NKC, HCOLS], F16, kind="ExternalInput").ap()
    wk16_d = nc.dram_tensor("wk16", [128, NKC, HCOLS], F16, kind="ExternalInput").ap()
    wv16_d = nc.dram_tensor("wv16", [128, NKC, HCOLS], F16, kind="ExternalInput").ap()
    wq8_d = nc.dram_tensor("wq8", [128, NKC, HCOLS], F8, kind="ExternalInput").ap()
    wk8_d = nc.dram_tensor("wk8", [128, NKC, HCOLS], F8, kind="ExternalInput").ap()
    wv8_d = nc.dram_tensor("wv8", [128, NKC, HCOLS], F8, kind="ExternalInput").ap()
    wo_d = nc.dram_tensor("wo", [128, 2, D], F16, kind="ExternalInput").ap()
    bq_d = nc.dram_tensor("bqp", [128, 2], F32, kind="ExternalInput").ap()
    mlo_d = nc.dram_tensor("mlo", [128, 128], F16, kind="ExternalInput").ap()
    mhi_d = nc.dram_tensor("mhi", [128, 128], F16, kind="ExternalInput").ap()
    out_d = nc.dram_tensor("out", [T, D], F16, kind="ExternalOutput").ap()

    with ExitStack() as ctx:
        const_pool = ctx.enter_context(tc.tile_pool(name="const", bufs=1))
        qk_pool = ctx.enter_context(tc.tile_pool(name="qk", bufs=1))
        w_pool = ctx.enter_context(tc.tile_pool(name="w", bufs=1))
        xt_pool = ctx.enter_context(tc.tile_pool(name="xt", bufs=3))
        pt_pool = ctx.enter_context(tc.tile_pool(name="pt", bufs=36))
        nrm_pool = ctx.enter_context(tc.tile_pool(name="nrm", bufs=4))
        stage_pool = ctx.enter_context(tc.tile_pool(name="stage", bufs=2))
        ps_s = ctx.enter_context(tc.tile_pool(name="ps_s", bufs=2, space="PSUM"))
        ps_pv = ctx.enter_context(tc.tile_pool(name="ps_pv", bufs=2, space="PSUM"))
        ps_mi = ctx.enter_context(tc.tile_pool(name="ps_mi", bufs=2, space="PSUM"))

        bq_sb = const_pool.tile([128, 2], F32)
        mask_lo = const_pool.tile([128, 128], F16)   # keep c >= p (upper incl)
        mask_hi = const_pool.tile([128, 128], F16)   # keep c < p (strict lower)
        warm = const_pool.tile([128, 640], F16)
        expw = const_pool.tile([128, 16], F16)

        wo_sb = qk_pool.tile([128, 2, D], F16)
        # V storage [j-part, J, head, 2*dk]; cols 0:64 of each head slot
        # are 1.0, so the PV matmul emits the softmax denominator already
        # broadcast across psum partitions 0:64.  v8 covers all J (fp8
        # stationary = hidden LDWEIGHTS); v16 duplicates J0..3 for the
        # precision-critical g0 PV.
        v8_sb = qk_pool.tile([128, NJ, HPC, 2 * DK], F8)
        v16_sb = qk_pool.tile([128, 4, HPC, 2 * DK], F16)
        q_sb = qk_pool.tile([128, 2, T], F16)
        k_sb = qk_pool.tile([128, 2, T], F16)
        osb16 = qk_pool.tile([128, 2, 512], F16)   # normalized O^T, g0
        osb8 = qk_pool.tile([128, 2, T], F8)       # normalized O^T, g1..3

        wq16_sb = w_pool.tile([128, NKC, HCOLS], F16)
        wk16_sb = w_pool.tile([128, NKC, HCOLS], F16)
        wv16_sb = w_pool.tile([128, NKC, HCOLS], F16)
        wq8_sb = w_pool.tile([128, NKC, HCOLS], F8)
        wk8_sb = w_pool.tile([128, NKC, HCOLS], F8)
        wv8_sb = w_pool.tile([128, NKC, HCOLS], F8)

        # ---- PE p-state + exp-table warmup (runs inside the DMA head) ----
        nc.vector.memset(warm[:], 0.0)
        nc.scalar.activation(expw[:], warm[:, 0:16],
                             mybir.ActivationFunctionType.Exp, scale=0.125)
        wtile = ps_mi.tile([128, 512], F32, tag="mi", name="warm")
        for _ in range(6):
            nc.tensor.matmul(wtile[:], warm[:, 0:128], warm[:, 128:640],
                             start=True, stop=True)

        # ---- x^T streamed by 512-column blocks ----
        # cb 0 is fp16 (precision-critical rows), cb 1..3 fp8.
        xt_tiles = {}

        def xt_dma(cb, engs=(None,)):
            xt_tiles[cb] = xt_pool.tile([128, NKC, 512], F8, tag="xt",
                                        name=f"xt_c{cb}")
            eng = engs[0] or nc.sync
            eng.dma_start(xt_tiles[cb][:, :, :], xT8_d[:, cb - 1, :, :])

        # prologue DMAs in strict first-use order, spread across the three
        # DMA-capable engines.  The first three tensors are split in k-halves
        # so u_q(0,0) starts after ~0.75MB instead of 2MB.
        nc.gpsimd.dma_start(bq_sb[:], bq_d[:, :])
        xt16 = xt_pool.tile([128, NKC, 512], F16, tag="xt16", name="xt_c0")
        xt_tiles[0] = xt16
        H = NKC // 2
        for half in range(2):
            ks = slice(half * H, (half + 1) * H)
            nc.sync.dma_start(xt16[:, ks, :], xT16_d[:, ks, :])
            nc.scalar.dma_start(wq16_sb[:, ks, :], wq16_d[:, ks, :])
            nc.gpsimd.dma_start(wk16_sb[:, ks, :], wk16_d[:, ks, :])
        nc.gpsimd.dma_start(mask_lo[:], mlo_d[:, :])
        nc.gpsimd.dma_start(mask_hi[:], mhi_d[:, :])
        xt_dma(1, engs=(nc.sync,))
        nc.scalar.dma_start(wq8_sb[:, :, :], wq8_d[:, :, :])
        nc.gpsimd.dma_start(wk8_sb[:, :, :], wk8_d[:, :, :])
        nc.scalar.dma_start(wv16_sb[:, :, :], wv16_d[:, :, :])
        xt_dma(2, engs=(nc.sync,))
        nc.gpsimd.dma_start(wv8_sb[:, :, :], wv8_d[:, :, :])
        nc.sync.dma_start(wo_sb[:, :, :], wo_d[:, :, :])
        # ones block of V_aug via memsets on prologue-idle engines
        nc.gpsimd.memset(v8_sb[:, 0:NJ // 2, :, 0:DK], 1.0)
        nc.vector.memset(v8_sb[:, NJ // 2:NJ, :, 0:DK], 1.0)
        nc.vector.memset(v16_sb[:, :, :, 0:DK], 1.0)

        # ---- projection units (~2-4k PE cycles each) ----
        def u_q(cb, m):
            xt = xt_tiles[cb]
            w_sb = wq16_sb if cb == 0 else wq8_sb
            qp = ps_mi.tile([128, 512], F32, tag="mi", name=f"qp{cb}{m}")
            for k in range(NKC):
                nc.tensor.matmul(
                    qp[:], w_sb[:, k, m * 128:(m + 1) * 128],
                    xt[:, k, :], start=(k == 0), stop=(k == NKC - 1),
                )
            nc.scalar.activation(
                q_sb[:, m, cb * 512:(cb + 1) * 512], qp[:],
                mybir.ActivationFunctionType.Identity,
                bias=bq_sb[:, m:m + 1],
                scale=(1.0 if cb == 0 else 1.0 / WS),
            )

        def u_k(cb, m):
            xt = xt_tiles[cb]
            w_sb = wk16_sb if cb == 0 else wk8_sb
            kp = ps_mi.tile([128, 512], F32, tag="mi", name=f"kp{cb}{m}")
            for k in range(NKC):
                nc.tensor.matmul(
                    kp[:], w_sb[:, k, m * 128:(m + 1) * 128],
                    xt[:, k, :], start=(k == 0), stop=(k == NKC - 1),
                )
            dst = k_sb[:, m, cb * 512:(cb + 1) * 512]
            if cb == 0:
                nc.vector.tensor_copy(dst, kp[:])
            else:
                nc.vector.tensor_scalar_mul(dst, kp[:], 1.0 / WS)

        def u_v(r):
            cb = r // 4
            xt = xt_tiles[cb]
            w_sb = wv16_sb if cb == 0 else wv8_sb
            vp = ps_mi.tile([128, HPC, DK], F32, tag="mi", name=f"vp{r}")
            for k in range(NKC):
                nc.tensor.matmul(
                    vp[:], xt[:, k, (r % 4) * 128:(r % 4) * 128 + 128],
                    w_sb[:, k, :], start=(k == 0), stop=(k == NKC - 1),
                )
            if cb == 0:
                nc.vector.tensor_copy(v16_sb[:, r, :, DK:2 * DK], vp[:])
                nc.scalar.copy(v8_sb[:, r, :, DK:2 * DK], vp[:])
            else:
                nc.vector.tensor_scalar_mul(
                    v8_sb[:, r, :, DK:2 * DK], vp[:], 1.0 / WS)

        # ---- attention ----
        # split into A (the wA S-matmul; allocates the psum + pt tiles) and
        # B (wB S-matmul + exp + masks) so the schedule can run A(J) a full
        # step before B(J): the exp drains while unrelated PE work streams,
        # and the J=0 A-part only needs cb0 (fp16) inputs while the fp8 x
        # chunks are still in flight.
        def attn_a(hpair, pts, J):
            wA = min(512, T - J * 128)
            for h in hpair:
                hp = slice((h % 2) * 64, (h % 2) * 64 + 64)
                hc = h // 2
                pt = pt_pool.tile([128, 640], F16, tag="pt",
                                  name=f"pt_h{h}_J{J}")
                pts[h][J] = pt
                s = ps_s.tile([128, 640], F32, tag="s", name=f"s_h{h}_J{J}")
                pts[h][(J, "s")] = s
                nc.tensor.matmul(
                    s[:, 0:wA], k_sb[hp, hc, J * 128:(J + 1) * 128],
                    q_sb[hp, hc, J * 128:J * 128 + wA],
                    start=True, stop=True,
                )

        def attn_b(hpair, pts, J):
            width = min(640, T - J * 128)
            wB = width - min(512, width)
            for h in hpair:
                hp = slice((h % 2) * 64, (h % 2) * 64 + 64)
                hc = h // 2
                s = pts[h].pop((J, "s"))
                pt = pts[h][J]
                if wB > 0:
                    nc.tensor.matmul(
                        s[:, 512:512 + wB],
                        k_sb[hp, hc, J * 128:(J + 1) * 128],
                        q_sb[hp, hc, J * 128 + 512:J * 128 + width],
                        start=True, stop=True,
                    )
                nc.scalar.activation(
                    pt[:, 0:width], s[:, 0:width],
                    mybir.ActivationFunctionType.Exp, scale=0.125,
                )
                nc.gpsimd.tensor_mul(pt[:, 0:128], pt[:, 0:128], mask_lo[:])
                if width == 640:
                    nc.gpsimd.tensor_mul(pt[:, 512:640], pt[:, 512:640],
                                         mask_hi[:])

        norm_state = {}

        def _pv_ranges(g, half):
            # column range, contributing J range, and pt tiles retired after.
            if half is None:
                return (512 * g, 512 * g + 512,
                        max(0, 4 * g - 4), 4 * g + 3,
                        range(max(0, 4 * g - 4), 4 * g))
            if half == "a":
                return (512 * g, 512 * g + 256,
                        4 * g - 4, 4 * g + 1,
                        range(4 * g - 4, 4 * g - 2))
            return (512 * g + 256, 512 * g + 512,
                    4 * g - 2, 4 * g + 3,
                    range(4 * g - 2, 4 * g + 4))

        def attn_pv(hpair, pts, g, half=None):
            collo, colhi, jlo, jhi, _ = _pv_ranges(g, half)
            v_sb = v16_sb if g == 0 else v8_sb
            pvs = {}
            for h in hpair:
                pv = ps_pv.tile([128, colhi - collo], F32, tag="pv",
                                name=f"pv_h{h}_g{g}{half or ''}")
                pvs[h] = pv
                jps = []
                for Jp in range(jlo, jhi + 1):
                    wJp = min(640, T - Jp * 128)
                    lo = max(Jp * 128, collo)
                    hi = min(Jp * 128 + wJp, colhi)
                    if hi > lo:
                        jps.append((Jp, lo, hi))
                # start=True lazily zeroes the whole psum bank; a full-width
                # contribution must come first
                jps.sort(key=lambda t: -(t[2] - t[1]))
                assert jps[0][2] - jps[0][1] == colhi - collo
                for idx, (Jp, lo, hi) in enumerate(jps):
                    nc.tensor.matmul(
                        pv[:, lo - collo:hi - collo],
                        v_sb[:, Jp, h, :],
                        pts[h][Jp][:, lo - Jp * 128:hi - Jp * 128],
                        start=(idx == 0), stop=(idx == len(jps) - 1),
                    )
            norm_state[(hpair, g, half)] = pvs
            for h in hpair:
                for Jp in _pv_ranges(g, half)[4]:
                    pts[h].pop(Jp, None)

        def attn_norm(hpair, g, half=None):
            # deferred ~1 step after attn_pv.  MUST be emitted before the pv
            # pool rotates into these tiles again (the osb multiply is the
            # psum tile's last reader).
            collo, colhi, _, _, _ = _pv_ranges(g, half)
            pvs = norm_state.pop((hpair, g, half))
            for h in hpair:
                rcp = nrm_pool.tile([64, colhi - collo], F32, tag="rcp",
                                    name=f"rcp_h{h}_g{g}{half or ''}")
                nc.vector.reciprocal_approx_fast(rcp[:], pvs[h][0:64, :])
                hp = slice((h % 2) * 64, (h % 2) * 64 + 64)
                if g == 0:
                    dst = osb16[hp, h // 2, 0:512]
                else:
                    dst = osb8[hp, h // 2, collo:colhi]
                nc.vector.tensor_mul(dst, pvs[h][64:128, :], rcp[:])

        def o_proj(qbs, alt_pool=False):
            # runs 1-2 steps after both pairs normalized the group; spread
            # across non-group steps so the mi-ring and the psum->SBUF copy
            # engines aren't slammed at group boundaries.  At the very tail
            # the S psum pool is retired, so its 4 banks double the po ring.
            for qb in qbs:
                so = stage_pool.tile([128, 1024], F16, tag="stage",
                                     name=f"so{qb}")
                for nh in range(2):
                    if alt_pool and nh == 1:
                        po = ps_s.tile([128, 512], F32, tag="s",
                                       name=f"po{qb}_{nh}")
                    else:
                        po = ps_mi.tile([128, 512], F32, tag="mi",
                                        name=f"po{qb}_{nh}")
                    for c in range(2):
                        if qb < 4:
                            stat = osb16[:, c, qb * 128:(qb + 1) * 128]
                        else:
                            stat = osb8[:, c, qb * 128:(qb + 1) * 128]
                        nc.tensor.matmul(
                            po[:], stat,
                            wo_sb[:, c, nh * 512:(nh + 1) * 512],
                            start=(c == 0), stop=(c == 1),
                        )
                    if nh == 0:
                        nc.scalar.copy(so[:, 0:512], po[:])
                    else:
                        nc.vector.tensor_copy(so[:, 512:1024], po[:])
                eng = (nc.sync, nc.scalar, nc.gpsimd)[qb % 3]
                eng.dma_start(out_d[qb * 128:(qb + 1) * 128, :], so[:, :])

        # ---- interleaved 19-step pipeline ----
        # Step s runs [B01(s-1), A01(s), B23(s-2), A23(s-1)] + fillers.
        # pair01's J=0 A-part needs only cb0 (fp16) data, so the PE starts
        # while the fp8 x chunks are still streaming; pair23 lags pair01 by
        # one J so the s-psum ring (2 bufs) alternates cleanly.  Group-3
        # PV/norm/o_proj are split in column halves (a: cols 1536:1792 from
        # J8..13, b: cols 1792:2048 from J10..15) so OP(12,13) overlaps the
        # final S blocks instead of serializing after them.
        pt01 = {0: {}, 1: {}}
        pt23 = {2: {}, 3: {}}
        P01, P23 = (0, 1), (2, 3)
        A01 = lambda J: attn_a(P01, pt01, J)
        B01 = lambda J: attn_b(P01, pt01, J)
        A23 = lambda J: attn_a(P23, pt23, J)
        B23 = lambda J: attn_b(P23, pt23, J)
        PV01 = lambda g, hf=None: attn_pv(P01, pt01, g, hf)
        PV23 = lambda g, hf=None: attn_pv(P23, pt23, g, hf)
        N01 = lambda g, hf=None: attn_norm(P01, g, hf)
        N23 = lambda g, hf=None: attn_norm(P23, g, hf)
        XT3 = lambda: xt_dma(3, engs=(nc.sync,))
        OP = lambda qbs, alt=False: o_proj(qbs, alt_pool=alt)

        u_q(0, 0)
        u_k(0, 0)

        # Fillers must be emitted no later than the step BEFORE the core op
        # that consumes them (PE executes in emission order): u_k(c,1) before
        # A23(4c) at s4c+1, u_q(3,*) before B01(8)/A23(9) at s9/s10, etc.
        FILL = {
            0:  [lambda: u_q(0, 1), lambda: u_k(0, 1), lambda: u_q(1, 0)],
            1:  [lambda: u_q(1, 1), lambda: u_v(0), lambda: u_v(1)],
            2:  [lambda: u_v(2), lambda: u_v(3), lambda: u_k(1, 0)],
            3:  [lambda: u_q(2, 0), XT3],
            4:  [lambda: PV01(0), lambda: u_k(1, 1)],
            5:  [lambda: N01(0), lambda: u_q(2, 1), lambda: u_v(4)],
            6:  [lambda: PV23(0), lambda: u_k(2, 0), lambda: u_v(5)],
            7:  [lambda: N23(0), lambda: OP([0, 1]), lambda: u_v(6),
                 lambda: u_q(3, 0)],
            8:  [lambda: u_v(7), lambda: PV01(1), lambda: u_k(2, 1)],
            9:  [lambda: N01(1), lambda: OP([2, 3]), lambda: u_v(8),
                 lambda: u_q(3, 1)],
            10: [lambda: PV23(1), lambda: u_v(9)],
            11: [lambda: N23(1), lambda: OP([4, 5]), lambda: u_k(3, 0),
                 lambda: u_v(10)],
            12: [lambda: u_v(11), lambda: PV01(2), lambda: u_k(3, 1)],
            13: [lambda: N01(2), lambda: OP([6, 7]), lambda: u_v(12)],
            14: [lambda: u_v(13), lambda: PV23(2), lambda: PV01(3, "a")],
            15: [lambda: N23(2), lambda: N01(3, "a"), lambda: OP([8, 9])],
            16: [lambda: u_v(14), lambda: u_v(15), lambda: PV23(3, "a"),
                 lambda: N23(3, "a"), lambda: OP([10, 11])],
            17: [lambda: OP([12, 13]), lambda: PV01(3, "b"),
                 lambda: N01(3, "b")],
            18: [lambda: PV23(3, "b"), lambda: N23(3, "b"),
                 lambda: OP([14, 15], True)],
        }

        for s in range(19):
            if 1 <= s <= 16:
                B01(s - 1)
            if s <= 15:
                A01(s)
            if 2 <= s <= 17:
                B23(s - 2)
            if 1 <= s <= 16:
                A23(s - 1)
            for f in FILL[s]:
                f()


def _build():
    if "nc" in _NC_CACHE:
        return _NC_CACHE["nc"]
    nc = bacc.Bacc("TRN2", debug=False)
    with tile.TileContext(nc) as tc:
        _emit(tc)
    nc.compile()
    _NC_CACHE["nc"] = nc
    return nc


def _sbt(a, dtype):
    """[nk*128, C] row-major -> SBUF partition-major [128, nk, C]."""
    nk = a.shape[0] // 128
    return np.ascontiguousarray(
        a.reshape(nk, 128, -1).transpose(1, 0, 2)).astype(dtype)


def _shard_inputs(x, Wq, bq, Wk, Wv, Wo):
    import ml_dtypes
    NP8 = ml_dtypes.float8_e4m3fn
    idx = np.arange(128)
    mlo = (idx[None, :] >= idx[:, None]).astype(np.float16)  # c >= p
    mhi = (idx[None, :] < idx[:, None]).astype(np.float16)   # c < p
    in_maps = []
    for b in range(2):
        xT = np.ascontiguousarray(x[b].T)
        xT16 = _sbt(xT[:, 0:512], np.float16)
        # [128, cb, k, 512] for the three fp8 column blocks
        xT8 = np.ascontiguousarray(
            np.stack([_sbt(xT[:, cb * 512:(cb + 1) * 512], NP8)
                      for cb in range(1, 4)], axis=1))
        for hg in range(4):
            cols = slice(hg * HCOLS, (hg + 1) * HCOLS)
            in_maps.append({
                "xT16": xT16,
                "xT8": xT8,
                "wq16": _sbt(Wq[:, cols], np.float16),
                "wk16": _sbt(Wk[:, cols], np.float16),
                "wv16": _sbt(Wv[:, cols], np.float16),
                "wq8": _sbt(Wq[:, cols] * WS, NP8),
                "wk8": _sbt(Wk[:, cols] * WS, NP8),
                "wv8": _sbt(Wv[:, cols] * WS, NP8),
                "wo": _sbt(Wo[cols, :], np.float16),
                "bqp": np.ascontiguousarray(bq[cols].reshape(2, 128).T),
                "mlo": mlo, "mhi": mhi,
            })
    return in_maps


def kernel(x, Wq, bq, Wk, bk, Wv, bv, Wo, bo, _trace=False, _tmpdir=None):
    x = np.asarray(x, dtype=np.float32)
    Wq = np.asarray(Wq, dtype=np.float32)
    Wk = np.asarray(Wk, dtype=np.float32)
    Wv = np.asarray(Wv, dtype=np.float32)
    Wo = np.asarray(Wo, dtype=np.float32)
    bq = np.asarray(bq, dtype=np.float32)
    bv = np.asarray(bv, dtype=np.float32)
    bo = np.asarray(bo, dtype=np.float32)

    nc = _build()
    in_maps = _shard_inputs(x, Wq, bq, Wk, Wv, Wo)
    res = run_bass_kernel_spmd(
        nc, in_maps, core_ids=list(range(8)), trace=_trace, tmpdir=_tmpdir,
    )
    host_bias = (bv @ Wo + bo).astype(np.float32)
    out = np.zeros((2, T, D), dtype=np.float32)
    for b in range(2):
        acc = res.results[b * 4]["out"].astype(np.float32).copy()
        for hg in range(1, 4):
            acc += res.results[b * 4 + hg]["out"]
        out[b] = acc + host_bias
    kernel._last_results = res
    return out
